# revision 22
# baseline (speedup 1.0000x reference)
"""Multi-head self-attention (AttnProcessor) on 8 Trainium2 NeuronCores.

B=1, S=4096, D=512, H=8 heads (head_dim=64). One head per core:
  core c computes  y_c = softmax((X Wq_c)(X Wk_c)^T / 8) (X Wv_c) Wo_c
with Wq_c = Wq[:, 64c:64c+64], Wo_c = Wo[64c:64c+64, :].
Host sums the 8 partial outputs and adds b_out.

Per-core layout (everything stays transposed so no on-chip transposes
are needed):
  ht  = X^T               [512, 4096]  (DMA'd in, host pre-transposed)
  qT  = Wq_c^T X^T        [64, 4096]   (d on partitions, s on free)
  kT  = Wk_c^T X^T        [64, 4096]
  vA  = [X Wv_c | 1]      [4096, 65]   (ones column -> softmax denominator)
  sT  = k-block x q-chunk [128, 512]   scores, k on partitions
  es  = exp(sT / 8)
  oT  = vA^T @ es         [65, 4096]   (row 64 = softmax denominators)
  y   = (oT[0:64].T @ Wo_c) * (1/denom)   [4096, 512]
Softmax max-subtraction is skipped: logits are ~N(0, 0.2), exp cannot
overflow.
"""

import numpy as np

S = 4096
D = 512
H = 8
HD = 64
NCORES = 8
NB = S // 128  # 32 s/k blocks of 128
NQ = S // 512  # 8 q chunks of 512
SS = 3  # k-blocks per superstep (one ScalarE exp instruction)

_CACHE = {}


def _build():
    import concourse.mybir as mybir
    from concourse import bacc
    from concourse.tile import TileContext

    f32 = mybir.dt.float32
    f32r = mybir.dt.float32r
    Exp = mybir.ActivationFunctionType.Exp

    nc = bacc.Bacc("TRN2", target_bir_lowering=False, debug=False, num_devices=NCORES)

    ht = nc.dram_tensor("ht", [D, S], f32r, kind="ExternalInput")
    wq = nc.dram_tensor("wq", [D, HD], f32r, kind="ExternalInput")
    wk = nc.dram_tensor("wk", [D, HD], f32r, kind="ExternalInput")
    wv = nc.dram_tensor("wv", [D, HD], f32r, kind="ExternalInput")
    wo = nc.dram_tensor("wo", [HD, D], f32r, kind="ExternalInput")
    y = nc.dram_tensor("y", [S, D], f32, kind="ExternalOutput")

    with TileContext(nc) as tc:
        with (
            tc.sbuf_pool(name="sb", bufs=1) as sb,
            tc.sbuf_pool(name="work", bufs=2) as work,
        ):
            # ---- weights first (small; must not queue behind the 8 MiB ht load)
            wq_sb = sb.tile([128, 4 * HD], f32r, name="wq_sb")
            wk_sb = sb.tile([128, 4 * HD], f32r, name="wk_sb")
            wv_sb = sb.tile([128, 4 * HD], f32r, name="wv_sb")
            for i in range(4):
                nc.sync.dma_start(
                    wq_sb[:, i * HD : (i + 1) * HD], wq[i * 128 : (i + 1) * 128, :]
                )
                nc.sync.dma_start(
                    wk_sb[:, i * HD : (i + 1) * HD], wk[i * 128 : (i + 1) * 128, :]
                )
                nc.sync.dma_start(
                    wv_sb[:, i * HD : (i + 1) * HD], wv[i * 128 : (i + 1) * 128, :]
                )
            wo_sb = sb.tile([HD, D], f32r, name="wo_sb")
            nc.sync.dma_start(wo_sb[:, :], wo[:, :])

            # column-major chunks: full 512-col groups land progressively so
            # kT consumption can chase the load
            ht_sb = sb.tile([128, 4 * S], f32r, name="ht_sb")
            for jj in range(4):
                for i in range(4):
                    nc.sync.dma_start(
                        ht_sb[:, i * S + jj * 1024 : i * S + (jj + 1) * 1024],
                        ht[i * 128 : (i + 1) * 128, jj * 1024 : (jj + 1) * 1024],
                    )

            ones = sb.tile([128, 1], f32, name="ones")
            nc.vector.memset(ones[:, :], 1.0)
            qT = sb.tile([HD, S], f32r, name="qT")
            kT = sb.tile([HD, S], f32r, name="kT")
            vA = sb.tile([128, NB * 65], f32r, name="vA")
            oT = sb.tile([65, S], f32r, name="oT")
            rc = sb.tile([128, NB], f32, name="rc")  # 1/denominator

            # ---- projections + attention, one PSUM pool ----
            # banks: s=6 (2x[128,1536]) + oT=1 + mm=1 -> 8
            with tc.psum_pool(name="ps", bufs=1) as ps:

                def qt_chunk(j, dst, w_sb):
                    pqk = ps.tile([HD, 512], f32, name="pqk", tag="mm", bufs=1)
                    for i in range(4):
                        nc.tensor.matmul(
                            pqk[:, :],
                            w_sb[:, i * HD : (i + 1) * HD],
                            ht_sb[:, i * S + j * 512 : i * S + (j + 1) * 512],
                            start=(i == 0),
                            stop=(i == 3),
                        )
                    nc.vector.tensor_copy(dst[:, j * 512 : (j + 1) * 512], pqk[:, :])

                def va_block(b):
                    psv = ps.tile([128, HD], f32, name="psv", tag="mm", bufs=1)
                    for i in range(4):
                        nc.tensor.matmul(
                            psv[:, :],
                            ht_sb[:, i * S + b * 128 : i * S + (b + 1) * 128],
                            wv_sb[:, i * HD : (i + 1) * HD],
                            start=(i == 0),
                            stop=(i == 3),
                        )
                    nc.vector.tensor_copy(vA[:, b * 65 : b * 65 + HD], psv[:, :])
                    nc.vector.tensor_copy(vA[:, b * 65 + HD : b * 65 + 65], ones[:, :])

                def proj(q):
                    # output projection + normalization for q's 4 row-blocks
                    for bb in range(4):
                        b = q * 4 + bb
                        bs = slice(b * 128, (b + 1) * 128)
                        py = ps.tile([128, D], f32, name="py", tag="mm", bufs=1)
                        nc.tensor.matmul(
                            py[:, :], oT[0:HD, bs], wo_sb[:, :], start=True, stop=True
                        )
                        y_sb = work.tile([128, D], f32, name="y_sb", tag="y", bufs=2)
                        nc.vector.tensor_scalar_mul(
                            y_sb[:, :], py[:, :], rc[:, b : b + 1]
                        )
                        nc.sync.dma_start(y[bs, :], y_sb[:, :])

                for j in range(4):
                    qt_chunk(j, kT, wk_sb)
                qt_chunk(0, qT, wq_sb)

                proj_pending = None
                for q in range(NQ):
                    qs = slice(q * 512, (q + 1) * 512)
                    poT = ps.tile([65, 512], f32, name="poT", tag="oT", bufs=1)
                    kb0 = 0
                    ss_idx = 0
                    while kb0 < NB:
                        w = min(SS, NB - kb0)
                        if q == 0:
                            for t in range(w):
                                va_block(kb0 + t)
                        pss = ps.tile(
                            [128, SS * 512], f32, name="pss", tag="s", bufs=2
                        )
                        for t in range(w):
                            kb = kb0 + t
                            nc.tensor.matmul(
                                pss[:, t * 512 : (t + 1) * 512],
                                kT[:, kb * 128 : (kb + 1) * 128],
                                qT[:, qs],
                                start=True,
                                stop=True,
                            )
                        es = work.tile(
                            [128, SS * 512], f32r, name="es", tag="es", bufs=3
                        )
                        nc.scalar.activation(
                            es[:, : w * 512], pss[:, : w * 512], Exp, scale=0.125
                        )
                        for t in range(w):
                            kb = kb0 + t
                            nc.tensor.matmul(
                                poT[:, :],
                                vA[:, kb * 65 : (kb + 1) * 65],
                                es[:, t * 512 : (t + 1) * 512],
                                start=(kb == 0),
                                stop=(kb == NB - 1),
                            )
                        kb0 += w
                        ss_idx += 1
                        if q == 0 and ss_idx == 2:
                            # second half of kT (its ht columns have landed by now)
                            for j in range(4, NQ):
                                qt_chunk(j, kT, wk_sb)
                        if ss_idx == 3:
                            # runway established: slot in next q's projections
                            # and the q+1 query chunk
                            if q + 1 < NQ:
                                qt_chunk(q + 1, qT, wq_sb)
                            if proj_pending is not None:
                                proj(proj_pending)
                                proj_pending = None
                    nc.vector.tensor_copy(oT[:, qs], poT[:, :])

                    # denominators: [1,128] rows -> [128,1] columns via a tiny
                    # K=1 PE matmul (dcol[p,0] = oT[64, bs][p] * 1)
                    dcol = ps.tile([128, 4], f32, name="dcol", tag="mm", bufs=1)
                    for bb in range(4):
                        b = q * 4 + bb
                        nc.tensor.matmul(
                            dcol[:, bb : bb + 1],
                            oT[64:65, b * 128 : (b + 1) * 128].bitcast(f32),
                            ones[64:65, 0:1],
                            start=True,
                            stop=True,
                        )
                    nc.vector.reciprocal(rc[:, q * 4 : q * 4 + 4], dcol[:, :])
                    proj_pending = q
                proj(proj_pending)

    nc.compile()
    return nc


def _get_nc():
    if "nc" not in _CACHE:
        _CACHE["nc"] = _build()
    return _CACHE["nc"]


def _make_in_maps(hidden_states, Wq, Wk, Wv, Wo):
    hT = np.ascontiguousarray(hidden_states.reshape(S, D).T).astype(np.float32)
    in_maps = []
    for c in range(NCORES):
        cs = slice(c * HD, (c + 1) * HD)
        in_maps.append(
            {
                "ht": hT,
                "wq": np.ascontiguousarray(Wq[:, cs]).astype(np.float32),
                "wk": np.ascontiguousarray(Wk[:, cs]).astype(np.float32),
                "wv": np.ascontiguousarray(Wv[:, cs]).astype(np.float32),
                "wo": np.ascontiguousarray(Wo[cs, :]).astype(np.float32),
            }
        )
    return in_maps


def kernel(hidden_states, Wq, Wk, Wv, Wo, b_out):
    from concourse.bass_utils import run_bass_kernel_spmd

    nc = _get_nc()
    in_maps = _make_in_maps(
        np.asarray(hidden_states, np.float32),
        np.asarray(Wq, np.float32),
        np.asarray(Wk, np.float32),
        np.asarray(Wv, np.float32),
        np.asarray(Wo, np.float32),
    )
    res = run_bass_kernel_spmd(nc, in_maps, list(range(NCORES)))
    acc = np.zeros((S, D), dtype=np.float64)
    for c in range(NCORES):
        acc += res.results[c]["y"].astype(np.float64)
    out = acc.astype(np.float32) + np.asarray(b_out, np.float32)[None, :]
    return out.reshape(1, S, D)


# revision 34
# speedup vs baseline: 21.7060x; 21.7060x over previous
"""Multi-head self-attention (AttnProcessor) on 8 Trainium2 NeuronCores.

B=1, S=4096, D=512, H=8 heads (head_dim=64). One head per core:
  core c computes  y_c = softmax((X Wq_c)(X Wk_c)^T / 8) (X Wv_c) Wo_c
with Wq_c = Wq[:, 64c:64c+64], Wo_c = Wo[64c:64c+64, :].
Host sums the 8 partial outputs and adds b_out.

Per-core layout (everything stays transposed so no on-chip transposes
are needed):
  ht  = X^T               [512, 4096]  (DMA'd in, host pre-transposed)
  qT  = Wq_c^T X^T        [64, 4096]   (d on partitions, s on free)
  kT  = Wk_c^T X^T        [64, 4096]
  vA  = [X Wv_c | 1]      [4096, 65]   (ones column -> softmax denominator)
  sT  = k-block x q-chunk [128, 512]   scores, k on partitions
  es  = exp(sT / 8)
  oT  = vA^T @ es         [65, 4096]   (row 64 = softmax denominators)
  y   = (oT[0:64].T @ Wo_c) * (1/denom)   [4096, 512]
Softmax max-subtraction is skipped: logits are ~N(0, 0.2), exp cannot
overflow.
"""

import numpy as np

S = 4096
D = 512
H = 8
HD = 64
NCORES = 8
NB = S // 128  # 32 s/k blocks of 128
NQ = S // 512  # 8 q chunks of 512
import os as _os

SS = int(_os.environ.get("KERNEL_SS", "2"))  # k-blocks per superstep
MMB = int(_os.environ.get("KERNEL_MMB", "2"))  # mm psum bufs
ESB = int(_os.environ.get("KERNEL_ESB", "3"))  # es sbuf bufs
YAGG = int(_os.environ.get("KERNEL_YAGG", "0"))  # aggregate y stores per chunk

_CACHE = {}


def _build(reps: int = 1):
    import concourse.mybir as mybir
    from concourse import bacc
    from concourse.tile import TileContext

    f32 = mybir.dt.float32
    f32r = mybir.dt.float32r
    Exp = mybir.ActivationFunctionType.Exp

    nc = bacc.Bacc("TRN2", target_bir_lowering=False, debug=False, num_devices=NCORES)

    ht = nc.dram_tensor("ht", [D, S], f32r, kind="ExternalInput")
    wq = nc.dram_tensor("wq", [D, HD], f32r, kind="ExternalInput")
    wk = nc.dram_tensor("wk", [D, HD], f32r, kind="ExternalInput")
    wv = nc.dram_tensor("wv", [D, HD], f32r, kind="ExternalInput")
    wo = nc.dram_tensor("wo", [HD, D], f32r, kind="ExternalInput")
    y = nc.dram_tensor("y", [S, D], f32, kind="ExternalOutput")

    with TileContext(nc) as tc:
        with (
            tc.sbuf_pool(name="sb", bufs=1) as sb,
            tc.sbuf_pool(name="work", bufs=2) as work,
        ):
            wq_sb = sb.tile([128, 4 * HD], f32r, name="wq_sb")
            wk_sb = sb.tile([128, 4 * HD], f32r, name="wk_sb")
            wv_sb = sb.tile([128, 4 * HD], f32r, name="wv_sb")
            wo_sb = sb.tile([HD, D], f32r, name="wo_sb")
            ht_sb = sb.tile([128, 4 * S], f32r, name="ht_sb")

            def load_inputs():
                # weights first (small; must not queue behind the 8 MiB ht load)
                for i in range(4):
                    nc.sync.dma_start(
                        wq_sb[:, i * HD : (i + 1) * HD], wq[i * 128 : (i + 1) * 128, :]
                    )
                    nc.sync.dma_start(
                        wk_sb[:, i * HD : (i + 1) * HD], wk[i * 128 : (i + 1) * 128, :]
                    )
                    nc.sync.dma_start(
                        wv_sb[:, i * HD : (i + 1) * HD], wv[i * 128 : (i + 1) * 128, :]
                    )
                nc.sync.dma_start(wo_sb[:, :], wo[:, :])
                # ht in column-major chunks: full 512-col groups land
                # progressively so kT consumption can chase the load
                for jj in range(4):
                    for i in range(4):
                        nc.sync.dma_start(
                            ht_sb[:, i * S + jj * 1024 : i * S + (jj + 1) * 1024],
                            ht[i * 128 : (i + 1) * 128, jj * 1024 : (jj + 1) * 1024],
                        )

            ones = sb.tile([128, 1], f32, name="ones")
            nc.vector.memset(ones[:, :], 1.0)
            qT = sb.tile([HD, S], f32r, name="qT")
            kT = sb.tile([HD, S], f32r, name="kT")
            vA = sb.tile([128, NB * 65], f32r, name="vA")
            oT = sb.tile([65, S], f32r, name="oT")
            rc = sb.tile([128, NB], f32, name="rc")  # 1/denominator

            # ---- projections + attention, one PSUM pool ----
            # banks: s=4 (2x[128,1024]) + oT=1 + mm=2 -> 7
            with tc.psum_pool(name="ps", bufs=1) as ps:
              for _rep in range(reps):
                  load_inputs()

                  def qt_chunk(j, dst, w_sb):
                      pqk = ps.tile([HD, 512], f32, name="pqk", tag="mm", bufs=MMB)
                      for i in range(4):
                          nc.tensor.matmul(
                              pqk[:, :],
                              w_sb[:, i * HD : (i + 1) * HD],
                              ht_sb[:, i * S + j * 512 : i * S + (j + 1) * 512],
                              start=(i == 0),
                              stop=(i == 3),
                          )
                      nc.vector.tensor_copy(dst[:, j * 512 : (j + 1) * 512], pqk[:, :])

                  def va_block(b):
                      psv = ps.tile([128, HD], f32, name="psv", tag="mm", bufs=MMB)
                      for i in range(4):
                          nc.tensor.matmul(
                              psv[:, :],
                              ht_sb[:, i * S + b * 128 : i * S + (b + 1) * 128],
                              wv_sb[:, i * HD : (i + 1) * HD],
                              start=(i == 0),
                              stop=(i == 3),
                          )
                      nc.vector.tensor_copy(vA[:, b * 65 : b * 65 + HD], psv[:, :])
                      nc.vector.tensor_copy(vA[:, b * 65 + HD : b * 65 + 65], ones[:, :])

                  def proj(q):
                      # output projection + normalization for q's 4 row-blocks
                      if YAGG:
                          y_sb = work.tile(
                              [128, 4 * D], f32, name="y_sb", tag="y", bufs=2
                          )
                      for bb in range(4):
                          b = q * 4 + bb
                          bs = slice(b * 128, (b + 1) * 128)
                          py = ps.tile([128, D], f32, name="py", tag="mm", bufs=MMB)
                          nc.tensor.matmul(
                              py[:, :], oT[0:HD, bs], wo_sb[:, :], start=True, stop=True
                          )
                          if YAGG:
                              nc.vector.tensor_scalar_mul(
                                  y_sb[:, bb * D : (bb + 1) * D],
                                  py[:, :],
                                  rc[:, b : b + 1],
                              )
                          else:
                              y_sb = work.tile(
                                  [128, D], f32, name="y_sb", tag="y", bufs=2
                              )
                              nc.vector.tensor_scalar_mul(
                                  y_sb[:, :], py[:, :], rc[:, b : b + 1]
                              )
                              nc.sync.dma_start(y[bs, :], y_sb[:, :])
                      if YAGG:
                          # one aggregated 1 MiB store for the whole chunk
                          y_view = y[q * 512 : (q + 1) * 512, :].rearrange(
                              "(b p) d -> p (b d)", p=128
                          )
                          nc.sync.dma_start(y_view, y_sb[:, :])

                  for j in range(4):
                      qt_chunk(j, kT, wk_sb)
                  qt_chunk(0, qT, wq_sb)

                  proj_pending = None
                  for q in range(NQ):
                      qs = slice(q * 512, (q + 1) * 512)
                      poT = ps.tile([65, 512], f32, name="poT", tag="oT", bufs=2)
                      kb0 = 0
                      ss_idx = 0
                      while kb0 < NB:
                          w = min(SS, NB - kb0)
                          if q == 0:
                              for t in range(w):
                                  va_block(kb0 + t)
                          pss = ps.tile(
                              [128, SS * 512], f32, name="pss", tag="s", bufs=2
                          )
                          for t in range(w):
                              kb = kb0 + t
                              nc.tensor.matmul(
                                  pss[:, t * 512 : (t + 1) * 512],
                                  kT[:, kb * 128 : (kb + 1) * 128],
                                  qT[:, qs],
                                  start=True,
                                  stop=True,
                              )
                          es = work.tile(
                              [128, SS * 512], f32r, name="es", tag="es", bufs=ESB
                          )
                          nc.scalar.activation(
                              es[:, : w * 512], pss[:, : w * 512], Exp, scale=0.125
                          )
                          for t in range(w):
                              kb = kb0 + t
                              nc.tensor.matmul(
                                  poT[:, :],
                                  vA[:, kb * 65 : (kb + 1) * 65],
                                  es[:, t * 512 : (t + 1) * 512],
                                  start=(kb == 0),
                                  stop=(kb == NB - 1),
                              )
                          kb0 += w
                          ss_idx += 1
                          if q == 0 and ss_idx == 4:
                              # second half of kT (its ht columns have landed by now)
                              for j in range(4, NQ):
                                  qt_chunk(j, kT, wk_sb)
                          if ss_idx == 3:
                              # runway established: slot in next q's projections
                              # and the q+1 query chunk
                              if q + 1 < NQ:
                                  qt_chunk(q + 1, qT, wq_sb)
                              if proj_pending is not None:
                                  proj(proj_pending)
                                  proj_pending = None
                      nc.vector.tensor_copy(oT[:, qs], poT[:, :])

                      # denominators: [1,128] rows -> [128,1] columns via a tiny
                      # K=1 PE matmul (dcol[p,0] = oT[64, bs][p] * 1)
                      dcol = ps.tile([128, 4], f32, name="dcol", tag="mm", bufs=MMB)
                      for bb in range(4):
                          b = q * 4 + bb
                          nc.tensor.matmul(
                              dcol[:, bb : bb + 1],
                              oT[64:65, b * 128 : (b + 1) * 128].bitcast(f32),
                              ones[64:65, 0:1],
                              start=True,
                              stop=True,
                          )
                      nc.vector.reciprocal(rc[:, q * 4 : q * 4 + 4], dcol[:, :])
                      proj_pending = q
                  proj(proj_pending)

    nc.compile()
    return nc


def _get_nc(reps: int = 1):
    key = ("nc", reps)
    if key not in _CACHE:
        _CACHE[key] = _build(reps)
    return _CACHE[key]


def _make_in_maps(hidden_states, Wq, Wk, Wv, Wo):
    hT = np.ascontiguousarray(hidden_states.reshape(S, D).T).astype(np.float32)
    in_maps = []
    for c in range(NCORES):
        cs = slice(c * HD, (c + 1) * HD)
        in_maps.append(
            {
                "ht": hT,
                "wq": np.ascontiguousarray(Wq[:, cs]).astype(np.float32),
                "wk": np.ascontiguousarray(Wk[:, cs]).astype(np.float32),
                "wv": np.ascontiguousarray(Wv[:, cs]).astype(np.float32),
                "wo": np.ascontiguousarray(Wo[cs, :]).astype(np.float32),
            }
        )
    return in_maps


def kernel(hidden_states, Wq, Wk, Wv, Wo, b_out):
    from concourse.bass_utils import run_bass_kernel_spmd

    nc = _get_nc()
    in_maps = _make_in_maps(
        np.asarray(hidden_states, np.float32),
        np.asarray(Wq, np.float32),
        np.asarray(Wk, np.float32),
        np.asarray(Wv, np.float32),
        np.asarray(Wo, np.float32),
    )
    res = run_bass_kernel_spmd(nc, in_maps, list(range(NCORES)))
    acc = np.zeros((S, D), dtype=np.float64)
    for c in range(NCORES):
        acc += res.results[c]["y"].astype(np.float64)
    out = acc.astype(np.float32) + np.asarray(b_out, np.float32)[None, :]
    return out.reshape(1, S, D)

